# revision 9
# baseline (speedup 1.0000x reference)
"""PointTransformerLayer on 8 trn2 NeuronCores.

Strategy (data-parallel over nodes, 6250 nodes/core):
  - Host folds all BatchNorm scales/biases into the linear weights and
    builds a fused per-node input fhatT = [features^T; points^T; ones]
    (fp16).
  - Device builds a global node table G[N, 256] fp16 =
    [-kg~ | -pp~ | v | qg~] (kg~ = (f@Wk+bk)@Wg1*A2 etc.) with one
    matmul per 128-node tile, stores to HBM scratch.
  - Per 128-edge subtile one indirect DMA (128 rows, one per partition)
    gathers G rows for the neighbor indices, edge-major [128e, 256ch].
    The gathered tile is the matmul *stationary*, so the transpose into
    the channel-major PSUM accumulator is free:
       PSUM bank H rows 0:64   h1pre = -kg^T + qg_b + p @ Wg1~
                  rows 64:128  pp-chain, later reused for w1 = v^T + p
    Per-node broadcast terms enter via a selector matmul (S32).
    ACT runs the two folded BN+relu stages and exp; DVE the attention
    multiply and grouped (K=16) reductions.  bg2 is dropped (softmax
    shift invariance).
  - Tail: r/s -> normalize, Wo projection, PE transpose, store.
"""

import sys
import numpy as np

sys.path.insert(0, "/opt/trn_rl_repo")

import concourse.bass as bass
import concourse.mybir as mybir
import concourse.tile as tile
import concourse.bass_utils as _bu

N = 50000
C = 64
K = 16
EPS = 1e-5
N_CORES = 8

NODES_PER_CORE = 6250
NODES_PAD = 6272           # 49 * 128
N_TAB = 50048              # 391 * 128  (>= 43750 + 6272 for core 7's own slice)

_ENGINE_CACHE = {}


def _enable_dynamic_dge():
    """This walrus build disables DynamicDMA by default; indirect DMAs
    (vector dynamic offsets) need the dge-levels flags."""
    if getattr(_bu, "_dge_patched", False):
        return
    orig = _bu.bir_verify_and_optimise

    def patched(tmpdir, inp="bir.json", outp="file.neff", arch=None, *, dve_root=None):
        saved = _bu.run_command

        def run_patched(argv, **kw):
            if argv and "walrus_driver" in str(argv[0]) and "--pass" in argv:
                argv = list(argv) + [
                    "--dge-levels=io",
                    "--dge-levels=spill_reload",
                    "--dge-levels=scalar_dynamic_offset",
                    "--dge-levels=vector_dynamic_offsets",
                ]
            return saved(argv, **kw)

        _bu.run_command = run_patched
        try:
            return orig(tmpdir, inp, outp, arch, dve_root=dve_root)
        finally:
            _bu.run_command = saved

    _bu.bir_verify_and_optimise = patched
    _bu._dge_patched = True


def _split_extra_waits(nc, max_waits=1):
    """This walrus build rejects >2 sync waits on one instruction; hoist
    extras onto preceding EventSemaphore no-ops."""
    n = 0
    for f in nc.m.functions:
        for b in f.blocks:
            newlist = []
            for ins in b.instructions:
                si = ins.sync_info
                if si is not None and len(si.on_wait) > max_waits:
                    extra = list(si.on_wait[:-max_waits])
                    keep = list(si.on_wait[-max_waits:])
                    for w in extra:
                        nop = mybir.InstEventSemaphore(
                            name=f"I-splitwait-{n}", ins=[], outs=[],
                            sync_info=mybir.SyncInfo(on_wait=[w], on_update=[]))
                        n += 1
                        nop.engine = ins.engine
                        newlist.append(nop)
                    si.on_wait = keep
                newlist.append(ins)
            b.instructions[:] = newlist
    return n


def _host_fold(inputs):
    f32 = np.float32
    Wq, bq = inputs["Wq"].astype(f32), inputs["bq"].astype(f32)
    Wk, bk = inputs["Wk"].astype(f32), inputs["bk"].astype(f32)
    Wv, bv = inputs["Wv"].astype(f32), inputs["bv"].astype(f32)
    Wp, b_p = inputs["Wp"].astype(f32), inputs["b_p"].astype(f32)
    Wg1, bg1 = inputs["Wg1"].astype(f32), inputs["bg1"].astype(f32)
    Wg2 = inputs["Wg2"].astype(f32)
    Wo, bo = inputs["Wo"].astype(f32), inputs["bo"].astype(f32)

    A1 = inputs["bnp_g"].astype(f32) / np.sqrt(inputs["bnp_v"].astype(f32) + EPS)
    B1 = inputs["bnp_b"].astype(f32) - inputs["bnp_m"].astype(f32) * A1 + b_p * A1
    A2 = inputs["bng_g"].astype(f32) / np.sqrt(inputs["bng_v"].astype(f32) + EPS)
    B2 = inputs["bng_b"].astype(f32) - inputs["bng_m"].astype(f32) * A2 + bg1 * A2

    Wp_t = Wp * A1[None, :]
    Wg1_t = Wg1 * A2[None, :]
    Wkg = Wk @ Wg1_t
    bkg = bk @ Wg1_t
    Wqg = Wq @ Wg1_t
    bqg = bq @ Wg1_t

    n_tab = _shapes["n_tab"]
    n = _shapes["n"]
    points = inputs["points"].astype(f32)
    features = inputs["features"].astype(f32)
    fhatT = np.zeros((68, n_tab), np.float16)
    fhatT[0:64, :n] = features.T.astype(np.float16)
    fhatT[64:67, :n] = points.T.astype(np.float16)
    fhatT[67, :n] = 1.0

    What_G = np.zeros((68, 256), np.float16)
    What_G[0:64, 0:64] = (-Wkg).astype(np.float16)
    What_G[67, 0:64] = (-bkg).astype(np.float16)
    What_G[64:67, 64:128] = (-Wp_t).astype(np.float16)
    What_G[0:64, 128:192] = Wv.astype(np.float16)
    What_G[67, 128:192] = bv.astype(np.float16)
    What_G[0:64, 192:256] = Wqg.astype(np.float16)
    What_G[67, 192:256] = bqg.astype(np.float16)

    WhatO = np.zeros((68, 128), np.float16)
    WhatO[0:64, 0:64] = Wqg.astype(np.float16)
    WhatO[67, 0:64] = bqg.astype(np.float16)
    WhatO[64:67, 64:128] = Wp_t.astype(np.float16)

    W1I = np.zeros((64, 128), np.float16)
    W1I[:, 0:64] = Wg1_t.astype(np.float16)
    W1I[:, 64:128] = np.eye(64, dtype=np.float16)

    Wg2pad = np.zeros((64, 128), np.float16)
    Wg2pad[:, 64:128] = Wg2.astype(np.float16)

    Wo_pad = np.zeros((128, 64), f32)
    Wo_pad[64:128, :] = Wo

    S32 = np.zeros((128, 512), np.float16)
    for j in range(32):
        S32[j, j * 16:(j + 1) * 16] = 1.0
    S32[32:64] = S32[0:32]; S32[64:96] = S32[0:32]; S32[96:128] = S32[0:32]

    I128 = np.eye(128, dtype=np.float16)
    I64f = np.zeros((128, 64), f32)
    I64f[0:64, :] = np.eye(64, dtype=f32)

    B1v = np.zeros((128, 1), f32); B1v[64:128, 0] = B1
    B2v = np.zeros((128, 1), f32); B2v[0:64, 0] = B2
    bov = np.zeros((128, 1), f32); bov[0:64, 0] = bo

    return dict(fhatT=fhatT, What_G=What_G, WhatO=WhatO, W1I=W1I,
                Wg2pad=Wg2pad, Wo_pad=Wo_pad, S32=S32, I128=I128, I64f=I64f,
                B1v=B1v, B2v=B2v, bov=bov)


# overridable for small sim tests
_shapes = dict(n=N, n_tab=N_TAB, nodes_per_core=NODES_PER_CORE,
               nodes_pad=NODES_PAD)


def _build_nc(split_waits=True):
    dt = mybir.dt
    n_tab = _shapes["n_tab"]
    nodes_pad = _shapes["nodes_pad"]
    edges_pad = nodes_pad * K
    n_sub = edges_pad // 128           # 128-edge subtiles
    n_grp = n_sub // 4                 # 512-edge PSUM bank groups
    own_tiles = nodes_pad // 128
    n_tiles_tab = n_tab // 128

    nc = bass.Bass()
    fhatT = nc.dram_tensor("fhatT", [68, n_tab], dt.float16, kind="ExternalInput")
    fhatT_own = nc.dram_tensor("fhatT_own", [68, nodes_pad], dt.float16, kind="ExternalInput")
    idx_in = nc.dram_tensor("idx", [128, n_sub], dt.int32, kind="ExternalInput")
    What_G = nc.dram_tensor("What_G", [68, 256], dt.float16, kind="ExternalInput")
    WhatO = nc.dram_tensor("WhatO", [68, 128], dt.float16, kind="ExternalInput")
    W1I_in = nc.dram_tensor("W1I", [64, 128], dt.float16, kind="ExternalInput")
    Wg2_in = nc.dram_tensor("Wg2pad", [64, 128], dt.float16, kind="ExternalInput")
    Wo_in = nc.dram_tensor("Wo_pad", [128, 64], dt.float32, kind="ExternalInput")
    S32_in = nc.dram_tensor("S32", [128, 512], dt.float16, kind="ExternalInput")
    I128_in = nc.dram_tensor("I128", [128, 128], dt.float16, kind="ExternalInput")
    I64f_in = nc.dram_tensor("I64f", [128, 64], dt.float32, kind="ExternalInput")
    B1_in = nc.dram_tensor("B1v", [128, 1], dt.float32, kind="ExternalInput")
    B2_in = nc.dram_tensor("B2v", [128, 1], dt.float32, kind="ExternalInput")
    bo_in = nc.dram_tensor("bov", [128, 1], dt.float32, kind="ExternalInput")

    G = nc.dram_tensor("G", [n_tab, 256], dt.float16)
    out = nc.dram_tensor("out", [nodes_pad, 64], dt.float32, kind="ExternalOutput")

    with tile.TileContext(nc) as tc:
        with (
            tc.tile_pool(name="const", bufs=1) as cpool,
            tc.tile_pool(name="work", bufs=3) as wrk,
            tc.tile_pool(name="accum", bufs=1) as acc,
        ):
            # ---- constants ----
            whatg = cpool.tile([68, 256], dt.float16)
            whato = cpool.tile([68, 128], dt.float16)
            w1i_hi = cpool.tile([128, 128], dt.float16)  # rows 64:128 = [Wg1~|I]
            wg2 = cpool.tile([64, 128], dt.float16)
            wo = cpool.tile([128, 64], dt.float32)
            s32 = cpool.tile([128, 512], dt.float16)
            i128 = cpool.tile([128, 128], dt.float16)
            i64f = cpool.tile([128, 64], dt.float32)
            b1 = cpool.tile([128, 1], dt.float32)
            b2 = cpool.tile([128, 1], dt.float32)
            bo = cpool.tile([128, 1], dt.float32)
            idx_sb = cpool.tile([128, n_sub], dt.int32)
            o_own = cpool.tile([128, own_tiles * 128], dt.float16)

            nc.sync.dma_start(out=whatg[:], in_=What_G[:])
            nc.sync.dma_start(out=whato[:], in_=WhatO[:])
            nc.sync.dma_start(out=w1i_hi[64:128, :], in_=W1I_in[:])
            nc.sync.dma_start(out=wg2[:], in_=Wg2_in[:])
            nc.sync.dma_start(out=wo[:], in_=Wo_in[:])
            nc.sync.dma_start(out=s32[:], in_=S32_in[:])
            nc.sync.dma_start(out=i128[:], in_=I128_in[:])
            nc.sync.dma_start(out=i64f[:], in_=I64f_in[:])
            nc.sync.dma_start(out=b1[:], in_=B1_in[:])
            nc.sync.dma_start(out=b2[:], in_=B2_in[:])
            nc.sync.dma_start(out=bo[:], in_=bo_in[:])
            nc.sync.dma_start(out=idx_sb[:], in_=idx_in[:])

            # ---- phase 1: build G (and O for own shard) ----
            with tc.tile_pool(name="p1ps", bufs=2, space="PSUM") as pp1, \
                 tc.tile_pool(name="p1sb", bufs=3) as gpool:
                for t in range(n_tiles_tab):
                    fh = gpool.tile([68, 128], dt.float16, tag="fh")
                    nc.sync.dma_start(out=fh[:], in_=fhatT[:, t * 128:(t + 1) * 128])
                    gp = pp1.tile([128, 256], dt.float32, tag="gp")
                    nc.tensor.matmul(gp[:], lhsT=fh[:], rhs=whatg[:],
                                     start=True, stop=True)
                    gh = gpool.tile([128, 256], dt.float16, tag="gh")
                    if t % 2 == 0:
                        nc.scalar.copy(out=gh[:], in_=gp[:])
                    else:
                        nc.vector.tensor_copy(out=gh[:], in_=gp[:])
                    nc.sync.dma_start(out=G[t * 128:(t + 1) * 128, :], in_=gh[:])
                for t in range(own_tiles):
                    fh = gpool.tile([68, 128], dt.float16, tag="fh")
                    nc.sync.dma_start(out=fh[:],
                                      in_=fhatT_own[:, t * 128:(t + 1) * 128])
                    op_ = pp1.tile([128, 128], dt.float32, tag="op")
                    nc.tensor.matmul(op_[:], lhsT=fh[:], rhs=whato[:],
                                     start=True, stop=True)
                    nc.vector.tensor_copy(out=o_own[:, t * 128:(t + 1) * 128],
                                          in_=op_[:])

            # ---- phase 2: main edge loop ----
            racc = acc.tile([128, n_grp * 32], dt.float32)   # rows 64:128 live
            sacc = acc.tile([128, n_grp * 32], dt.float32)

            with tc.tile_pool(name="p2ps", bufs=2, space="PSUM") as pp2, \
                 tc.tile_pool(name="p2g", bufs=8) as gat:
                for g in range(n_grp):
                    H = pp2.tile([128, 512], dt.float32, tag="H")
                    D = pp2.tile([128, 512], dt.float32, tag="D")
                    W = pp2.tile([128, 512], dt.float32, tag="W")
                    gts = []
                    for s in range(4):
                        st = g * 4 + s
                        gt = gat.tile([128, 256], dt.float16, tag="gt")
                        nc.gpsimd.indirect_dma_start(
                            out=gt[:], out_offset=None, in_=G[:],
                            in_offset=bass.IndirectOffsetOnAxis(
                                ap=idx_sb[:, st:st + 1], axis=0))
                        gts.append(gt)
                    for s in range(4):
                        sl = slice(s * 128, (s + 1) * 128)
                        # P1: inject [-kg^T ; -pp^T]
                        nc.tensor.matmul(H[:, sl], lhsT=gts[s][:, 0:128],
                                         rhs=i128[:], start=(s == 0), stop=False,
                                         skip_group_check=True)
                    # P2: + [qg_b ; ppb_b] for the whole 512-edge group
                    n0 = g * 32
                    base = n0 % 128
                    chb = (n0 // 128) * 128
                    tp = (base, 0) if base else None
                    nc.tensor.matmul(H[:], lhsT=o_own[base:base + 32, chb:chb + 128],
                                     rhs=s32[base:base + 32, :], start=False,
                                     stop=False, tile_position=tp,
                                     skip_group_check=True)
                    # ACT-1: p = relu(-H_bot + B1)
                    p_t = wrk.tile([128, 512], dt.float16, tag="p")
                    nc.scalar.activation(p_t[64:128, :], H[64:128, :],
                                         mybir.ActivationFunctionType.Relu,
                                         bias=b1[64:128, :], scale=-1.0)
                    for s in range(4):
                        sl = slice(s * 128, (s + 1) * 128)
                        # P3: w1 accumulation bank W rows 64:128 <- v^T
                        nc.tensor.matmul(W[64:128, sl], lhsT=gts[s][:, 128:192],
                                         rhs=i128[:], start=(s == 0), stop=False,
                                         tile_position=(0, 64),
                                         skip_group_check=True)
                    for s in range(4):
                        sl = slice(s * 128, (s + 1) * 128)
                        # P4a: h1pre += p @ Wg1~
                        nc.tensor.matmul(H[0:64, sl],
                                         lhsT=w1i_hi[64:128, 0:64],
                                         rhs=p_t[64:128, sl], start=False,
                                         stop=True, tile_position=(64, 0),
                                         skip_group_check=True)
                        # P4b: w1 += p
                        nc.tensor.matmul(W[64:128, sl],
                                         lhsT=w1i_hi[64:128, 64:128],
                                         rhs=p_t[64:128, sl], start=False,
                                         stop=True, tile_position=(64, 64),
                                         skip_group_check=True)
                    # ACT-2: h1r = relu(H_top + B2)
                    h1r = wrk.tile([64, 512], dt.float16, tag="h1r")
                    nc.scalar.activation(h1r[:], H[0:64, :],
                                         mybir.ActivationFunctionType.Relu,
                                         bias=b2[0:64, :], scale=1.0)
                    for s in range(4):
                        sl = slice(s * 128, (s + 1) * 128)
                        # P5: h2 -> D rows 64:128
                        nc.tensor.matmul(D[:, sl], lhsT=wg2[:], rhs=h1r[:, sl],
                                         start=(s == 0), stop=(s == 3),
                                         skip_group_check=True)
                    # ACT-3: eh = exp(h2)
                    eh = wrk.tile([128, 512], dt.float32, tag="eh")
                    nc.scalar.activation(eh[64:128, :], D[64:128, :],
                                         mybir.ActivationFunctionType.Exp)
                    # DVE: w2 = w1 * eh ; grouped reductions over K=16
                    w2 = wrk.tile([128, 512], dt.float32, tag="w2")
                    nc.vector.tensor_tensor(out=w2[64:128, :], in0=W[64:128, :],
                                            in1=eh[64:128, :],
                                            op=mybir.AluOpType.mult)
                    nc.vector.tensor_reduce(
                        out=racc[64:128, g * 32:(g + 1) * 32],
                        in_=w2[64:128, :].rearrange("p (n k) -> p n k", k=K),
                        axis=mybir.AxisListType.X, op=mybir.AluOpType.add)
                    nc.vector.tensor_reduce(
                        out=sacc[64:128, g * 32:(g + 1) * 32],
                        in_=eh[64:128, :].rearrange("p (n k) -> p n k", k=K),
                        axis=mybir.AxisListType.X, op=mybir.AluOpType.add)

            # ---- phase 3: normalize, Wo, transpose, store ----
            with tc.tile_pool(name="p3ps", bufs=2, space="PSUM") as pp3:
                rec = acc.tile([128, n_grp * 32], dt.float32)
                nc.vector.reciprocal(rec[64:128, :], sacc[64:128, :])
                r2 = acc.tile([128, n_grp * 32], dt.float32)
                nc.vector.tensor_tensor(out=r2[64:128, :], in0=racc[64:128, :],
                                        in1=rec[64:128, :],
                                        op=mybir.AluOpType.mult)
                for t in range(own_tiles):
                    sl = slice(t * 128, (t + 1) * 128)
                    ob = pp3.tile([64, 128], dt.float32, tag="ob")
                    nc.tensor.matmul(ob[:], lhsT=wo[64:128, :],
                                     rhs=r2[64:128, sl], start=True, stop=True)
                    o1 = wrk.tile([64, 128], dt.float32, tag="o1")
                    nc.scalar.activation(o1[:], ob[:],
                                         mybir.ActivationFunctionType.Identity,
                                         bias=bo[0:64, :], scale=1.0)
                    ot2 = pp3.tile([128, 64], dt.float32, tag="ot2")
                    nc.tensor.matmul(ot2[:], lhsT=o1[:], rhs=i64f[0:64, :],
                                     start=True, stop=True)
                    o2 = wrk.tile([128, 64], dt.float32, tag="o2")
                    nc.vector.tensor_copy(out=o2[:], in_=ot2[:])
                    nc.sync.dma_start(out=out[t * 128:(t + 1) * 128, :], in_=o2[:])

    nc.finalize()
    if split_waits:
        _split_extra_waits(nc)
    return nc


def _prep_in_maps(inputs, folded, n_cores):
    gi = inputs["grouped_indices"]
    npc = _shapes["nodes_per_core"]
    nodes_pad = _shapes["nodes_pad"]
    edges_pad = nodes_pad * K
    in_maps = []
    for c in range(n_cores):
        lo = c * npc
        own_idx = np.asarray(gi[lo:lo + npc]).astype(np.int64)
        idx_flat = np.zeros(edges_pad, np.int32)
        idx_flat[:npc * K] = own_idx.reshape(-1).astype(np.int32)
        idx_wrapped = np.ascontiguousarray(
            idx_flat.reshape(edges_pad // 128, 128).T)
        fho = np.ascontiguousarray(folded["fhatT"][:, lo:lo + nodes_pad])
        in_maps.append(dict(
            fhatT=folded["fhatT"], fhatT_own=fho, idx=idx_wrapped,
            What_G=folded["What_G"], WhatO=folded["WhatO"], W1I=folded["W1I"],
            Wg2pad=folded["Wg2pad"], Wo_pad=folded["Wo_pad"], S32=folded["S32"],
            I128=folded["I128"], I64f=folded["I64f"], B1v=folded["B1v"],
            B2v=folded["B2v"], bov=folded["bov"]))
    return in_maps


def kernel(**inputs) -> np.ndarray:
    _enable_dynamic_dge()
    folded = _host_fold(inputs)
    if "main" not in _ENGINE_CACHE:
        _ENGINE_CACHE["main"] = _build_nc()
    nc = _ENGINE_CACHE["main"]
    in_maps = _prep_in_maps(inputs, folded, N_CORES)
    res = _bu.run_bass_kernel_spmd(nc, in_maps, core_ids=list(range(N_CORES)))
    npc = _shapes["nodes_per_core"]
    outs = [res.results[c]["out"][:npc] for c in range(N_CORES)]
    return np.concatenate(outs, axis=0)[:_shapes["n"]].astype(np.float32)


# revision 12
# speedup vs baseline: 1.0951x; 1.0951x over previous
"""PointTransformerLayer on 8 trn2 NeuronCores.

Strategy (data-parallel over nodes, 6250 nodes/core):
  - Host folds all BatchNorm scales/biases into the linear weights and
    builds a fused per-node input fhatT = [features^T; points^T; ones]
    (fp16).
  - Device builds a global node table G[N, 256] fp16 =
    [-kg~ | -pp~ | v | qg~] (kg~ = (f@Wk+bk)@Wg1*A2 etc.) with one
    matmul per 128-node tile, stores to HBM scratch.
  - Per 128-edge subtile one indirect DMA (128 rows, one per partition)
    gathers G rows for the neighbor indices, edge-major [128e, 256ch].
    The gathered tile is the matmul *stationary*, so the transpose into
    the channel-major PSUM accumulator is free:
       PSUM bank H rows 0:64   h1pre = -kg^T + qg_b + p @ Wg1~
                  rows 64:128  pp-chain, later reused for w1 = v^T + p
    Per-node broadcast terms enter via a selector matmul (S32).
    ACT runs the two folded BN+relu stages and exp; DVE the attention
    multiply and grouped (K=16) reductions.  bg2 is dropped (softmax
    shift invariance).
  - Tail: r/s -> normalize, Wo projection, PE transpose, store.
"""

import sys
import numpy as np

sys.path.insert(0, "/opt/trn_rl_repo")

import concourse.bass as bass
import concourse.mybir as mybir
import concourse.tile as tile
import concourse.bass_utils as _bu

N = 50000
C = 64
K = 16
EPS = 1e-5
N_CORES = 8

NODES_PER_CORE = 6250
NODES_PAD = 6272           # 49 * 128
N_TAB = 50048              # 391 * 128  (>= 43750 + 6272 for core 7's own slice)

_ENGINE_CACHE = {}


def _enable_dynamic_dge():
    """This walrus build disables DynamicDMA by default; indirect DMAs
    (vector dynamic offsets) need the dge-levels flags."""
    if getattr(_bu, "_dge_patched", False):
        return
    orig = _bu.bir_verify_and_optimise

    def patched(tmpdir, inp="bir.json", outp="file.neff", arch=None, *, dve_root=None):
        saved = _bu.run_command

        def run_patched(argv, **kw):
            if argv and "walrus_driver" in str(argv[0]) and "--pass" in argv:
                argv = list(argv) + [
                    "--dge-levels=io",
                    "--dge-levels=spill_reload",
                    "--dge-levels=scalar_dynamic_offset",
                    "--dge-levels=vector_dynamic_offsets",
                ]
            return saved(argv, **kw)

        _bu.run_command = run_patched
        try:
            return orig(tmpdir, inp, outp, arch, dve_root=dve_root)
        finally:
            _bu.run_command = saved

    _bu.bir_verify_and_optimise = patched
    _bu._dge_patched = True


def _split_extra_waits(nc, max_waits=1):
    """This walrus build rejects >2 sync waits on one instruction; hoist
    extras onto preceding EventSemaphore no-ops."""
    n = 0
    for f in nc.m.functions:
        for b in f.blocks:
            newlist = []
            for ins in b.instructions:
                si = ins.sync_info
                if si is not None and len(si.on_wait) > max_waits:
                    extra = list(si.on_wait[:-max_waits])
                    keep = list(si.on_wait[-max_waits:])
                    for w in extra:
                        nop = mybir.InstEventSemaphore(
                            name=f"I-splitwait-{n}", ins=[], outs=[],
                            sync_info=mybir.SyncInfo(on_wait=[w], on_update=[]))
                        n += 1
                        nop.engine = ins.engine
                        newlist.append(nop)
                    si.on_wait = keep
                newlist.append(ins)
            b.instructions[:] = newlist
    return n


def _host_fold(inputs):
    f32 = np.float32
    Wq, bq = inputs["Wq"].astype(f32), inputs["bq"].astype(f32)
    Wk, bk = inputs["Wk"].astype(f32), inputs["bk"].astype(f32)
    Wv, bv = inputs["Wv"].astype(f32), inputs["bv"].astype(f32)
    Wp, b_p = inputs["Wp"].astype(f32), inputs["b_p"].astype(f32)
    Wg1, bg1 = inputs["Wg1"].astype(f32), inputs["bg1"].astype(f32)
    Wg2 = inputs["Wg2"].astype(f32)
    Wo, bo = inputs["Wo"].astype(f32), inputs["bo"].astype(f32)

    A1 = inputs["bnp_g"].astype(f32) / np.sqrt(inputs["bnp_v"].astype(f32) + EPS)
    B1 = inputs["bnp_b"].astype(f32) - inputs["bnp_m"].astype(f32) * A1 + b_p * A1
    A2 = inputs["bng_g"].astype(f32) / np.sqrt(inputs["bng_v"].astype(f32) + EPS)
    B2 = inputs["bng_b"].astype(f32) - inputs["bng_m"].astype(f32) * A2 + bg1 * A2

    Wp_t = Wp * A1[None, :]
    Wg1_t = Wg1 * A2[None, :]
    Wkg = Wk @ Wg1_t
    bkg = bk @ Wg1_t
    Wqg = Wq @ Wg1_t
    bqg = bq @ Wg1_t

    n_tab = _shapes["n_tab"]
    n = _shapes["n"]
    points = inputs["points"].astype(f32)
    features = inputs["features"].astype(f32)
    fhatT = np.zeros((68, n_tab), np.float16)
    fhatT[0:64, :n] = features.T.astype(np.float16)
    fhatT[64:67, :n] = points.T.astype(np.float16)
    fhatT[67, :n] = 1.0

    What_G = np.zeros((68, 256), np.float16)
    What_G[0:64, 0:64] = (-Wkg).astype(np.float16)
    What_G[67, 0:64] = (-bkg).astype(np.float16)
    What_G[64:67, 64:128] = (-Wp_t).astype(np.float16)
    What_G[0:64, 128:192] = Wv.astype(np.float16)
    What_G[67, 128:192] = bv.astype(np.float16)
    What_G[0:64, 192:256] = Wqg.astype(np.float16)
    What_G[67, 192:256] = bqg.astype(np.float16)

    WhatO = np.zeros((68, 128), np.float16)
    WhatO[0:64, 0:64] = Wqg.astype(np.float16)
    WhatO[67, 0:64] = bqg.astype(np.float16)
    WhatO[64:67, 64:128] = Wp_t.astype(np.float16)

    W1I = np.zeros((64, 128), np.float16)
    W1I[:, 0:64] = Wg1_t.astype(np.float16)
    W1I[:, 64:128] = np.eye(64, dtype=np.float16)

    Wg2pad = np.zeros((64, 128), np.float16)
    Wg2pad[:, 64:128] = Wg2.astype(np.float16)

    Wo_pad = np.zeros((128, 64), f32)
    Wo_pad[64:128, :] = Wo

    S32 = np.zeros((128, 512), np.float16)
    for j in range(32):
        S32[j, j * 16:(j + 1) * 16] = 1.0
    S32[32:64] = S32[0:32]; S32[64:96] = S32[0:32]; S32[96:128] = S32[0:32]

    I128 = np.eye(128, dtype=np.float16)
    I64f = np.zeros((128, 64), f32)
    I64f[0:64, :] = np.eye(64, dtype=f32)

    B1v = np.zeros((128, 1), f32); B1v[64:128, 0] = B1
    B2v = np.zeros((128, 1), f32); B2v[0:64, 0] = B2
    bov = np.zeros((128, 1), f32); bov[0:64, 0] = bo

    return dict(fhatT=fhatT, What_G=What_G, WhatO=WhatO, W1I=W1I,
                Wg2pad=Wg2pad, Wo_pad=Wo_pad, S32=S32, I128=I128, I64f=I64f,
                B1v=B1v, B2v=B2v, bov=bov)


# overridable for small sim tests
_shapes = dict(n=N, n_tab=N_TAB, nodes_per_core=NODES_PER_CORE,
               nodes_pad=NODES_PAD)


def _build_nc(split_waits=True):
    dt = mybir.dt
    n_tab = _shapes["n_tab"]
    nodes_pad = _shapes["nodes_pad"]
    edges_pad = nodes_pad * K
    n_sub = edges_pad // 128           # 128-edge subtiles
    n_grp = n_sub // 4                 # 512-edge PSUM bank groups
    own_tiles = nodes_pad // 128
    n_tiles_tab = n_tab // 128

    nc = bass.Bass()
    fhatT = nc.dram_tensor("fhatT", [68, n_tab], dt.float16, kind="ExternalInput")
    fhatT_own = nc.dram_tensor("fhatT_own", [68, nodes_pad], dt.float16, kind="ExternalInput")
    idx_in = nc.dram_tensor("idx", [128, n_sub], dt.int32, kind="ExternalInput")
    What_G = nc.dram_tensor("What_G", [68, 256], dt.float16, kind="ExternalInput")
    WhatO = nc.dram_tensor("WhatO", [68, 128], dt.float16, kind="ExternalInput")
    W1I_in = nc.dram_tensor("W1I", [64, 128], dt.float16, kind="ExternalInput")
    Wg2_in = nc.dram_tensor("Wg2pad", [64, 128], dt.float16, kind="ExternalInput")
    Wo_in = nc.dram_tensor("Wo_pad", [128, 64], dt.float32, kind="ExternalInput")
    S32_in = nc.dram_tensor("S32", [128, 512], dt.float16, kind="ExternalInput")
    I128_in = nc.dram_tensor("I128", [128, 128], dt.float16, kind="ExternalInput")
    I64f_in = nc.dram_tensor("I64f", [128, 64], dt.float32, kind="ExternalInput")
    B1_in = nc.dram_tensor("B1v", [128, 1], dt.float32, kind="ExternalInput")
    B2_in = nc.dram_tensor("B2v", [128, 1], dt.float32, kind="ExternalInput")
    bo_in = nc.dram_tensor("bov", [128, 1], dt.float32, kind="ExternalInput")

    G = nc.dram_tensor("G", [n_tab, 256], dt.float16)
    out = nc.dram_tensor("out", [nodes_pad, 64], dt.float32, kind="ExternalOutput")

    with tile.TileContext(nc) as tc:
        with (
            tc.tile_pool(name="const", bufs=1) as cpool,
            tc.tile_pool(name="work", bufs=6) as wrk,
            tc.tile_pool(name="accum", bufs=1) as acc,
        ):
            # ---- constants ----
            whatg = cpool.tile([68, 256], dt.float16)
            whato = cpool.tile([68, 128], dt.float16)
            w1i_hi = cpool.tile([128, 128], dt.float16)  # rows 64:128 = [Wg1~|I]
            wg2 = cpool.tile([64, 128], dt.float16)
            wo = cpool.tile([128, 64], dt.float32)
            s32 = cpool.tile([128, 512], dt.float16)
            i128 = cpool.tile([128, 128], dt.float16)
            i64f = cpool.tile([128, 64], dt.float32)
            b1 = cpool.tile([128, 1], dt.float32)
            b2 = cpool.tile([128, 1], dt.float32)
            bo = cpool.tile([128, 1], dt.float32)
            idx_sb = cpool.tile([128, n_sub], dt.int32)
            o_own = cpool.tile([128, own_tiles * 128], dt.float16)

            nc.sync.dma_start(out=whatg[:], in_=What_G[:])
            nc.sync.dma_start(out=whato[:], in_=WhatO[:])
            nc.sync.dma_start(out=w1i_hi[64:128, :], in_=W1I_in[:])
            nc.sync.dma_start(out=wg2[:], in_=Wg2_in[:])
            nc.sync.dma_start(out=wo[:], in_=Wo_in[:])
            nc.sync.dma_start(out=s32[:], in_=S32_in[:])
            nc.sync.dma_start(out=i128[:], in_=I128_in[:])
            nc.sync.dma_start(out=i64f[:], in_=I64f_in[:])
            nc.sync.dma_start(out=b1[:], in_=B1_in[:])
            nc.sync.dma_start(out=b2[:], in_=B2_in[:])
            nc.sync.dma_start(out=bo[:], in_=bo_in[:])
            nc.sync.dma_start(out=idx_sb[:], in_=idx_in[:])

            # ---- phase 1: build G (and O for own shard) ----
            with tc.tile_pool(name="p1ps", bufs=2, space="PSUM") as pp1, \
                 tc.tile_pool(name="p1sb", bufs=3) as gpool:
                for t in range(n_tiles_tab):
                    fh = gpool.tile([68, 128], dt.float16, tag="fh")
                    nc.sync.dma_start(out=fh[:], in_=fhatT[:, t * 128:(t + 1) * 128])
                    gp = pp1.tile([128, 256], dt.float32, tag="gp")
                    nc.tensor.matmul(gp[:], lhsT=fh[:], rhs=whatg[:],
                                     start=True, stop=True)
                    gh = gpool.tile([128, 256], dt.float16, tag="gh")
                    if t % 2 == 0:
                        nc.scalar.copy(out=gh[:], in_=gp[:])
                    else:
                        nc.vector.tensor_copy(out=gh[:], in_=gp[:])
                    nc.sync.dma_start(out=G[t * 128:(t + 1) * 128, :], in_=gh[:])
                for t in range(own_tiles):
                    fh = gpool.tile([68, 128], dt.float16, tag="fh")
                    nc.sync.dma_start(out=fh[:],
                                      in_=fhatT_own[:, t * 128:(t + 1) * 128])
                    op_ = pp1.tile([128, 128], dt.float32, tag="op")
                    nc.tensor.matmul(op_[:], lhsT=fh[:], rhs=whato[:],
                                     start=True, stop=True)
                    nc.vector.tensor_copy(out=o_own[:, t * 128:(t + 1) * 128],
                                          in_=op_[:])

            # ---- phase 2: main edge loop ----
            racc = acc.tile([128, n_grp * 32], dt.float32)   # rows 64:128 live
            sacc = acc.tile([128, n_grp * 32], dt.float32)

            with tc.tile_pool(name="p2ps", bufs=2, space="PSUM") as pp2, \
                 tc.tile_pool(name="p2g", bufs=24) as gat:
                for g in range(n_grp):
                    H = pp2.tile([128, 512], dt.float32, tag="H")
                    D = pp2.tile([128, 512], dt.float32, tag="D")
                    W = pp2.tile([128, 512], dt.float32, tag="W")
                    gts = []
                    for s in range(4):
                        st = g * 4 + s
                        gt = gat.tile([128, 256], dt.float16, tag="gt")
                        nc.gpsimd.indirect_dma_start(
                            out=gt[:], out_offset=None, in_=G[:],
                            in_offset=bass.IndirectOffsetOnAxis(
                                ap=idx_sb[:, st:st + 1], axis=0))
                        gts.append(gt)
                    for s in range(4):
                        sl = slice(s * 128, (s + 1) * 128)
                        # P1: inject [-kg^T ; -pp^T]
                        nc.tensor.matmul(H[:, sl], lhsT=gts[s][:, 0:128],
                                         rhs=i128[:], start=(s == 0), stop=False,
                                         skip_group_check=True)
                    # P2: + [qg_b ; ppb_b] for the whole 512-edge group
                    n0 = g * 32
                    base = n0 % 128
                    chb = (n0 // 128) * 128
                    tp = (base, 0) if base else None
                    nc.tensor.matmul(H[:], lhsT=o_own[base:base + 32, chb:chb + 128],
                                     rhs=s32[base:base + 32, :], start=False,
                                     stop=False, tile_position=tp,
                                     skip_group_check=True)
                    # ACT-1: p = relu(-H_bot + B1)
                    p_t = wrk.tile([128, 512], dt.float16, tag="p")
                    nc.scalar.activation(p_t[64:128, :], H[64:128, :],
                                         mybir.ActivationFunctionType.Relu,
                                         bias=b1[64:128, :], scale=-1.0)
                    for s in range(4):
                        sl = slice(s * 128, (s + 1) * 128)
                        # P3: w1 accumulation bank W rows 64:128 <- v^T
                        nc.tensor.matmul(W[64:128, sl], lhsT=gts[s][:, 128:192],
                                         rhs=i128[:], start=(s == 0), stop=False,
                                         tile_position=(0, 64),
                                         skip_group_check=True)
                    for s in range(4):
                        sl = slice(s * 128, (s + 1) * 128)
                        # P4a: h1pre += p @ Wg1~
                        nc.tensor.matmul(H[0:64, sl],
                                         lhsT=w1i_hi[64:128, 0:64],
                                         rhs=p_t[64:128, sl], start=False,
                                         stop=True, tile_position=(64, 0),
                                         skip_group_check=True)
                        # P4b: w1 += p
                        nc.tensor.matmul(W[64:128, sl],
                                         lhsT=w1i_hi[64:128, 64:128],
                                         rhs=p_t[64:128, sl], start=False,
                                         stop=True, tile_position=(64, 64),
                                         skip_group_check=True)
                    # ACT-2: h1r = relu(H_top + B2)
                    h1r = wrk.tile([64, 512], dt.float16, tag="h1r")
                    nc.scalar.activation(h1r[:], H[0:64, :],
                                         mybir.ActivationFunctionType.Relu,
                                         bias=b2[0:64, :], scale=1.0)
                    for s in range(4):
                        sl = slice(s * 128, (s + 1) * 128)
                        # P5: h2 -> D rows 64:128
                        nc.tensor.matmul(D[:, sl], lhsT=wg2[:], rhs=h1r[:, sl],
                                         start=(s == 0), stop=(s == 3),
                                         skip_group_check=True)
                    # ACT-3: eh = exp(h2)
                    eh = wrk.tile([128, 512], dt.float32, tag="eh")
                    nc.scalar.activation(eh[64:128, :], D[64:128, :],
                                         mybir.ActivationFunctionType.Exp)
                    # DVE: w2 = w1 * eh ; grouped reductions over K=16
                    w2 = wrk.tile([128, 512], dt.float32, tag="w2")
                    nc.vector.tensor_tensor(out=w2[64:128, :], in0=W[64:128, :],
                                            in1=eh[64:128, :],
                                            op=mybir.AluOpType.mult)
                    nc.vector.tensor_reduce(
                        out=racc[64:128, g * 32:(g + 1) * 32],
                        in_=w2[64:128, :].rearrange("p (n k) -> p n k", k=K),
                        axis=mybir.AxisListType.X, op=mybir.AluOpType.add)
                    nc.vector.tensor_reduce(
                        out=sacc[64:128, g * 32:(g + 1) * 32],
                        in_=eh[64:128, :].rearrange("p (n k) -> p n k", k=K),
                        axis=mybir.AxisListType.X, op=mybir.AluOpType.add)

            # ---- phase 3: normalize, Wo, transpose, store ----
            with tc.tile_pool(name="p3ps", bufs=2, space="PSUM") as pp3:
                rec = acc.tile([128, n_grp * 32], dt.float32)
                nc.vector.reciprocal(rec[64:128, :], sacc[64:128, :])
                r2 = acc.tile([128, n_grp * 32], dt.float32)
                nc.vector.tensor_tensor(out=r2[64:128, :], in0=racc[64:128, :],
                                        in1=rec[64:128, :],
                                        op=mybir.AluOpType.mult)
                for t in range(own_tiles):
                    sl = slice(t * 128, (t + 1) * 128)
                    ob = pp3.tile([64, 128], dt.float32, tag="ob")
                    nc.tensor.matmul(ob[:], lhsT=wo[64:128, :],
                                     rhs=r2[64:128, sl], start=True, stop=True)
                    o1 = wrk.tile([64, 128], dt.float32, tag="o1")
                    nc.scalar.activation(o1[:], ob[:],
                                         mybir.ActivationFunctionType.Identity,
                                         bias=bo[0:64, :], scale=1.0)
                    ot2 = pp3.tile([128, 64], dt.float32, tag="ot2")
                    nc.tensor.matmul(ot2[:], lhsT=o1[:], rhs=i64f[0:64, :],
                                     start=True, stop=True)
                    o2 = wrk.tile([128, 64], dt.float32, tag="o2")
                    nc.vector.tensor_copy(out=o2[:], in_=ot2[:])
                    nc.sync.dma_start(out=out[t * 128:(t + 1) * 128, :], in_=o2[:])

    nc.finalize()
    if split_waits:
        _split_extra_waits(nc)
    return nc


def _prep_in_maps(inputs, folded, n_cores):
    gi = inputs["grouped_indices"]
    npc = _shapes["nodes_per_core"]
    nodes_pad = _shapes["nodes_pad"]
    edges_pad = nodes_pad * K
    in_maps = []
    for c in range(n_cores):
        lo = c * npc
        own_idx = np.asarray(gi[lo:lo + npc]).astype(np.int64)
        idx_flat = np.zeros(edges_pad, np.int32)
        idx_flat[:npc * K] = own_idx.reshape(-1).astype(np.int32)
        idx_wrapped = np.ascontiguousarray(
            idx_flat.reshape(edges_pad // 128, 128).T)
        fho = np.ascontiguousarray(folded["fhatT"][:, lo:lo + nodes_pad])
        in_maps.append(dict(
            fhatT=folded["fhatT"], fhatT_own=fho, idx=idx_wrapped,
            What_G=folded["What_G"], WhatO=folded["WhatO"], W1I=folded["W1I"],
            Wg2pad=folded["Wg2pad"], Wo_pad=folded["Wo_pad"], S32=folded["S32"],
            I128=folded["I128"], I64f=folded["I64f"], B1v=folded["B1v"],
            B2v=folded["B2v"], bov=folded["bov"]))
    return in_maps


def kernel(**inputs) -> np.ndarray:
    _enable_dynamic_dge()
    folded = _host_fold(inputs)
    if "main" not in _ENGINE_CACHE:
        _ENGINE_CACHE["main"] = _build_nc()
    nc = _ENGINE_CACHE["main"]
    in_maps = _prep_in_maps(inputs, folded, N_CORES)
    res = _bu.run_bass_kernel_spmd(nc, in_maps, core_ids=list(range(N_CORES)))
    npc = _shapes["nodes_per_core"]
    outs = [res.results[c]["out"][:npc] for c in range(N_CORES)]
    return np.concatenate(outs, axis=0)[:_shapes["n"]].astype(np.float32)
